# revision 1
# baseline (speedup 1.0000x reference)
"""Causal multi-head attention (B=4, S=2048, D=2048, H=16, RoPE) on 8 TRN2 NeuronCores.

Sharding: core c handles (batch b = c//2, head-group g = c%2) -- 8 heads per core.
Each core computes its head-group's Q/K/V projections (column-sharded weights),
RoPE, causal softmax attention, and the row-sharded Wo partial product.
The host sums the two partial outputs per batch (the "all-reduce") and
transposes back.

Device-side layout is fully transposed ("feature-major"): activations are kept
as [feature, seq] so every GEMM contracts over the partition dimension without
any on-device transposes. All matmul inputs are bf16 (fp32 accumulate in PSUM);
softmax runs in fp32.

Schedule (single core): K-proj -> V-proj -> per-head software pipeline
[Q-proj(h) || attention(h-1)] -> Wo. The pipeline keeps the PE dense while
the RoPE/exp epilogues of the previous head run on Vector/Scalar.
"""

import math
import sys
import types

import numpy as np
import ml_dtypes

BF16 = ml_dtypes.bfloat16

S = 2048
D = 2048
H = 16
DK = 128
B = 4
E = 1024          # head-group width (8 heads x 128)
HPC = 8           # heads per core
NT_D = 16         # 128-wide tiles along the contraction (model) dim
NT_S4 = 4         # 512-wide tiles along seq
NT_S16 = 16       # 128-wide tiles along seq
ROPE_THETA = 10000.0

# Set by test harness to capture a profile; kernel() then stores results here.
TRACE = False
LAST_RESULT = None

_PROGRAM_CACHE = {}


def _install_ntff_hook():
    """Register the NTFF profile hook that this image's antenv lacks.

    Only needed when TRACE=True; degrades silently if the axon .so predates
    NRT profiling.
    """
    if "antenv.axon_hooks" in sys.modules:
        return
    holder = {"hook": None}
    mod = types.ModuleType("antenv.axon_hooks")
    mod.set_axon_ntff_profile_hook = lambda h: holder.__setitem__("hook", h)
    mod.get_axon_ntff_profile_hook = lambda: holder["hook"]
    sys.modules["antenv.axon_hooks"] = mod
    try:
        from trn_agent_boot.trn_boot import _ntff_profile_via_ctypes

        mod.set_axon_ntff_profile_hook(
            _ntff_profile_via_ctypes("/opt/axon/libaxon_pjrt.so")
        )
    except Exception:
        pass


def _build_program():
    """Build + compile the single-core Bass program (same program on all 8 cores)."""
    if "nc" in _PROGRAM_CACHE:
        return _PROGRAM_CACHE["nc"]

    from contextlib import ExitStack

    import concourse.mybir as mybir
    import concourse.tile as tile
    from concourse import bacc

    F32 = mybir.dt.float32
    B16 = mybir.dt.bfloat16

    nc = bacc.Bacc("TRN2", target_bir_lowering=False, debug=False, num_devices=8)

    F16 = mybir.dt.float16

    xt = nc.dram_tensor("xt", [NT_D, 128, S], B16, kind="ExternalInput").ap()
    # Weights are host-prepped into partition-major contiguous blocks so every
    # load is a single 4KB-per-partition DMA (256B-element strided loads were
    # rate-limiting the startup).
    wq = nc.dram_tensor("wq", [HPC, 128, NT_D * 128], B16, kind="ExternalInput").ap()
    wk = nc.dram_tensor("wk", [HPC, 128, NT_D * 128], B16, kind="ExternalInput").ap()
    wv = nc.dram_tensor("wv", [NT_D, 128, E], B16, kind="ExternalInput").ap()
    wo = nc.dram_tensor("wo", [NT_D, 128, HPC * 128], B16, kind="ExternalInput").ap()
    cos = nc.dram_tensor("cos", [128, S], B16, kind="ExternalInput").ap()
    sin = nc.dram_tensor("sin", [128, S], B16, kind="ExternalInput").ap()
    msk = nc.dram_tensor("msk", [128, 512], B16, kind="ExternalInput").ap()
    ones = nc.dram_tensor("ones", [128, 128], B16, kind="ExternalInput").ap()
    # bf16 partial-output: the host sums the two per-batch partials in fp32;
    # the bf16 rounding of the partials is well inside the error budget and
    # halves the output DMA traffic.
    outt = nc.dram_tensor("outt", [D, S], B16, kind="ExternalOutput").ap()

    Exp = mybir.ActivationFunctionType.Exp

    with tile.TileContext(nc, pool_alloc_mode="queue") as tc, ExitStack() as ctx:
        cpool = ctx.enter_context(tc.tile_pool(name="const", bufs=1))

        # Persistent activation stores (bf16).
        kt_pool = ctx.enter_context(tc.tile_pool(name="kt", bufs=HPC))
        v_pool = ctx.enter_context(tc.tile_pool(name="v", bufs=NT_S16))
        kts, vts, ats = [], [], []

        # PSUM: proj tag (2 bufs) coexists with attention tags (6 bufs) = 8.
        # Opened inside the phase-1/2 scope below (closed before the Wo phase).
        ps_pool = None

        def load_w(wdram, e, nm, w_pool):
            wt = w_pool.tile([128, NT_D * 128], B16, tag="w", name=f"w{nm}_{e}")
            nc.sync.dma_start(out=wt, in_=wdram[e])
            return wt

        def rope_epilogue(ps, qh, s4, nm, e, r_pool):
            """RoPE: qh[:, sl] = ps*cos + swap_halves(ps)*sin.

            ACT does only the partition-crossing half-swap (DVE is
            lane-locked); the cos-mul reads the PSUM directly on DVE.
            """
            sl = slice(s4 * 512, (s4 + 1) * 512)
            qs = r_pool.tile([128, 512], B16, tag="u", name=f"qs_{nm}_{e}_{s4}")
            nc.scalar.copy(out=qs[0:64, :], in_=ps[64:128, :])
            nc.scalar.copy(out=qs[64:128, :], in_=ps[0:64, :])
            t1 = r_pool.tile([128, 512], B16, tag="t2", name=f"t1_{nm}_{e}_{s4}")
            u = r_pool.tile([128, 512], B16, tag="u2", name=f"u_{nm}_{e}_{s4}")
            nc.vector.tensor_mul(out=t1, in0=ps, in1=cos_t[:, sl])
            nc.vector.tensor_mul(out=u, in0=qs, in1=sin_t[:, sl])
            nc.vector.tensor_add(out=qh[:, sl], in0=t1, in1=u)

        def proj_pair(pool, pbufs, wt, qh, e, nm, sp, r_pool):
            """Emit one 1024-wide column group (2 psums) of a Q/K projection."""
            if True:
                psums = [
                    pool.tile([128, 512], F32, tag="p1", bufs=pbufs,
                              name=f"p{nm}_{e}_{sp}_{s2}")
                    for s2 in range(2)
                ]
                for d in range(NT_D):
                    lhsT = wt[:, d * 128 : (d + 1) * 128]
                    for s2 in range(2):
                        s4 = 2 * sp + s2
                        nc.tensor.matmul(
                            psums[s2], lhsT=lhsT,
                            rhs=xts[d][:, s4 * 512 : (s4 + 1) * 512],
                            start=(d == 0), stop=(d == NT_D - 1),
                        )
                for s2 in range(2):
                    s4 = 2 * sp + s2
                    rope_epilogue(psums[s2], qh, s4, nm, e, r_pool)

        def proj_qk(wdram, e, nm, out_pool, r_pool, w_pool, wt=None,
                    pool=None, pbufs=3):
            """One head's Q^T or K^T projection + RoPE; returns the bf16 tile."""
            if wt is None:
                wt = load_w(wdram, e, nm, w_pool)
            qh = out_pool.tile([128, S], B16, tag=nm + "t", name=f"{nm}h_{e}")
            for sp in range(2):
                proj_pair(pool, pbufs, wt, qh, e, nm, sp, r_pool)
            return qh

        def attention(h, qh, at, pt_pool, rc_pool, s4_range=None):
            """Causal attention for head h into at ([dv=128, S] bf16)."""
            for s4 in (range(NT_S4) if s4_range is None else s4_range):
                nsk = 4 * s4 + 4
                sl = slice(s4 * 512, (s4 + 1) * 512)
                pat = ps_pool.tile([128, 512], F32, tag="pat", bufs=2,
                                   name=f"pat_{h}_{s4}")
                den = ps_pool.tile([128, 512], F32, tag="den", bufs=2,
                                   name=f"den_{h}_{s4}")
                for sk in range(nsk):
                    # Diagonal tiles: columns j < 128*r are fully masked; skip
                    # them in all three matmuls. The first tile (sk==0) is
                    # always full width, so its start=True write covers the
                    # whole accumulation bank.
                    r = sk - 4 * s4
                    off = 128 * r if r > 0 else 0
                    w = 512 - off
                    psc_t = ps_pool.tile([128, 512], F32, tag="psc", bufs=2,
                                         name=f"psc_{h}_{s4}_{sk}")
                    psc = psc_t[:, 0:w]
                    nc.tensor.matmul(
                        psc,
                        lhsT=kts[h][:, sk * 128 : (sk + 1) * 128],
                        rhs=qh[:, s4 * 512 + off : (s4 + 1) * 512],
                        start=True, stop=True,
                    )
                    pt_t = pt_pool.tile([128, 512], B16, tag="pt",
                                        name=f"pt_{h}_{s4}_{sk}")
                    pt = pt_t[:, 0:w]
                    # Promote the exp+mask chain past the current head's
                    # RoPE epilogue ops in the Scalar/Vector streams: the
                    # attnV matmul needs the masked tile now, while RoPE
                    # results aren't consumed until the next head.
                    with tc.high_priority(offset=400):
                        nc.scalar.activation(out=pt, in_=psc, func=Exp)
                        if r >= 0:
                            nc.vector.tensor_mul(
                                out=pt, in0=pt, in1=msk_t[:, 0:w],
                            )
                    nc.tensor.matmul(
                        pat[:, off:512],
                        lhsT=vts[sk][:, h * 128 : (h + 1) * 128],
                        rhs=pt, start=(sk == 0), stop=(sk == nsk - 1),
                    )
                    nc.tensor.matmul(
                        den[:, off:512], lhsT=one_t, rhs=pt,
                        start=(sk == 0), stop=(sk == nsk - 1),
                    )
                rcb = rc_pool.tile([128, 512], F32, tag="rcb", bufs=2,
                                   name=f"rcb_{h}_{s4}")
                nc.vector.reciprocal_approx_fast(out=rcb, in_=den)
                nc.vector.tensor_mul(out=at[:, sl], in0=pat, in1=rcb)

        # ---------------- Phase 1a: load X^T, K projections ----------------
        with ExitStack() as p1ctx:
            xt_pool = p1ctx.enter_context(tc.tile_pool(name="xt", bufs=NT_D))
            w_pool = p1ctx.enter_context(tc.tile_pool(name="wst", bufs=2))
            r_pool = p1ctx.enter_context(tc.tile_pool(name="rope", bufs=2))
            # DMA priming order: tiny constants (warm-up data) first, then
            # K-head-0/1 weights, then X^T tiles with cos/sin/msk mid-stream.
            one_t = cpool.tile([128, 128], B16, tag="one", name="one_t")
            nc.sync.dma_start(out=one_t, in_=ones)
            wt0 = w_pool.tile([128, NT_D * 128], B16, tag="w", name="wk_0")
            for c in range(4):
                nc.sync.dma_start(
                    out=wt0[:, c * 512 : (c + 1) * 512],
                    in_=wk[0][:, c * 512 : (c + 1) * 512],
                )
            wt1 = load_w(wk, 1, "k", w_pool)
            xts = []
            for d in range(NT_D):
                xtile = xt_pool.tile([128, S], B16, tag="xt", name=f"xt_{d}")
                if d == 0:
                    for c in range(4):
                        nc.sync.dma_start(
                            out=xtile[:, c * 512 : (c + 1) * 512],
                            in_=xt[0][:, c * 512 : (c + 1) * 512],
                        )
                else:
                    nc.sync.dma_start(out=xtile, in_=xt[d])
                xts.append(xtile)
                if d == 7:
                    cos_t = cpool.tile([128, S], B16, tag="cos", name="cos_t")
                    nc.sync.dma_start(out=cos_t, in_=cos)
                    sin_t = cpool.tile([128, S], B16, tag="sin", name="sin_t")
                    nc.sync.dma_start(out=sin_t, in_=sin)
                    msk_t = cpool.tile([128, 512], B16, tag="msk",
                                       name="msk_t")
                    nc.sync.dma_start(out=msk_t, in_=msk)

            kv_ps = tc.alloc_tile_pool(name="kvps", bufs=8, space="PSUM")
            # 7 interleaved psum groups (K head 0 fully + head 1 s4<3) so the
            # PE consumes each arriving x^T tile at ~1.5us/tile, matching the
            # DMA rate of the initial 8MB x^T load; head 1's s4=3 group runs
            # after the d-loop (8th bank) to bridge the epilogue drain.
            groups = [(0, 0), (0, 1), (0, 2), (0, 3), (1, 0), (1, 1), (1, 2)]
            ps01 = {
                g: kv_ps.tile([128, 512], F32, tag="p1", bufs=8,
                              name=f"pk_{g[0]}_{g[1]}")
                for g in groups
            }
            ps13 = kv_ps.tile([128, 512], F32, tag="p1", bufs=8, name="pk_1_3")
            # ~3.5us of junk matmuls on the ones tile: sustains PE activity
            # through the HAM SHORT window so the real projections run at
            # 2.4GHz instead of the cold 1.2GHz default.
            for i in range(72):
                nc.tensor.matmul(
                    ps01[(0, 0)][:, 0:128], lhsT=one_t, rhs=one_t,
                    start=(i == 0), stop=(i == 71), skip_group_check=True,
                )
            pre_w = {0: wt0, 1: wt1}
            for d in range(NT_D):
                for e, s4 in groups:
                    nc.tensor.matmul(
                        ps01[(e, s4)],
                        lhsT=pre_w[e][:, d * 128 : (d + 1) * 128],
                        rhs=xts[d][:, s4 * 512 : (s4 + 1) * 512],
                        start=(d == 0), stop=(d == NT_D - 1),
                    )
            for d in range(NT_D):
                nc.tensor.matmul(
                    ps13, lhsT=wt1[:, d * 128 : (d + 1) * 128],
                    rhs=xts[d][:, 3 * 512 : 4 * 512],
                    start=(d == 0), stop=(d == NT_D - 1),
                )
            kh0 = kt_pool.tile([128, S], B16, tag="kt", name="kh_0")
            kh1 = kt_pool.tile([128, S], B16, tag="kt", name="kh_1")
            for e, s4 in groups:
                rope_epilogue(ps01[(e, s4)], kh0 if e == 0 else kh1,
                              s4, "k", e, r_pool)
            rope_epilogue(ps13, kh1, 3, "k", 1, r_pool)
            kts.append(kh0)
            kts.append(kh1)
            for e in range(2, HPC):
                kts.append(proj_qk(wk, e, "k", kt_pool, r_pool, w_pool,
                                   pool=kv_ps, pbufs=8))

            # ------------- Phase 1b: V projection -------------
            with tc.tile_pool(name="wvp", bufs=NT_D) as wv_pool:
                wvts = []
                for d in range(NT_D):
                    wvt = wv_pool.tile([128, E], B16, tag="wv", name=f"wv_{d}")
                    nc.sync.dma_start(out=wvt, in_=wv[d])
                    wvts.append(wvt)
                for s in range(NT_S16):
                    pv = [
                        kv_ps.tile([128, 512], F32, tag="p1", bufs=8,
                                   name=f"pv_{s}_{i}")
                        for i in range(2)
                    ]
                    for d in range(NT_D):
                        lhsT = xts[d][:, s * 128 : (s + 1) * 128]
                        for i in range(2):
                            nc.tensor.matmul(
                                pv[i], lhsT=lhsT,
                                rhs=wvts[d][:, i * 512 : (i + 1) * 512],
                                start=(d == 0), stop=(d == NT_D - 1),
                            )
                    vt = v_pool.tile([128, E], B16, tag="vt", name=f"vt_{s}")
                    nc.scalar.copy(out=vt[:, 0:512], in_=pv[0])
                    nc.scalar.copy(out=vt[:, 512:1024], in_=pv[1])
                    vts.append(vt)

            # ------- Phase 2: per-head pipeline: Q-proj(h) + attn(h-1) -------
            kv_ps.release()
            ps_pool = p1ctx.enter_context(
                tc.tile_pool(name="ps", bufs=2, space="PSUM")
            )
            attn_pool = ctx.enter_context(
                tc.tile_pool(name="attn", bufs=HPC, side="right")
            )
            qt_pool = p1ctx.enter_context(tc.tile_pool(name="qt", bufs=2))
            pt_pool = p1ctx.enter_context(tc.tile_pool(name="pt", bufs=5))
            rc_pool = p1ctx.enter_context(tc.tile_pool(name="rcp", bufs=2))

            def make_at(h):
                return attn_pool.tile([128, S], B16, tag="at", name=f"at_{h}")

            qhs = {}
            ats_t = {}
            for h in range(HPC):
                # Interleave the two q-proj column groups of head h with the
                # two attention halves of head h-1 so attention matmuls cover
                # the RoPE epilogue latency.
                wt = load_w(wq, h, "q", w_pool)
                qh = qt_pool.tile([128, S], B16, tag="qt", name=f"qh_{h}")
                qhs[h] = qh

                def q_group(s4):
                    ps = ps_pool.tile([128, 512], F32, tag="p1", bufs=2,
                                      name=f"pq_{h}_{s4}")
                    for d in range(NT_D):
                        nc.tensor.matmul(
                            ps, lhsT=wt[:, d * 128 : (d + 1) * 128],
                            rhs=xts[d][:, s4 * 512 : (s4 + 1) * 512],
                            start=(d == 0), stop=(d == NT_D - 1),
                        )
                    rope_epilogue(ps, qh, s4, "q", h, r_pool)

                q_group(0)
                q_group(1)
                if h > 0:
                    # s4 order puts mask-free full tiles first so the DVE can
                    # drain the RoPE queue before mask-muls are needed.
                    ats_t[h - 1] = make_at(h - 1)
                    attention(h - 1, qhs[h - 1], ats_t[h - 1], pt_pool,
                              rc_pool, s4_range=(1, 0))
                q_group(2)
                q_group(3)
                if h > 0:
                    attention(h - 1, qhs.pop(h - 1), ats_t[h - 1], pt_pool,
                              rc_pool, s4_range=(3, 2))
                    ats.append(ats_t[h - 1])
            wo_pool = ctx.enter_context(
                tc.tile_pool(name="wop", bufs=2, side="right")
            )

            def load_wo(eo):
                wot = wo_pool.tile([128, HPC * 128], B16, tag="wo",
                                   name=f"wo_{eo}")
                nc.sync.dma_start(out=wot, in_=wo[eo])
                return wot

            out_pool = ctx.enter_context(
                tc.tile_pool(name="outp", bufs=3, side="right")
            )

            def wo_emit(wot, psum, eo, s4, at7=None):
                """One (eo, s4) Wo accumulation chain + evacuate + DMA out.

                at7: head-7's at tile while it is still being produced (tail
                interleave); head 7 is accumulated LAST so the first 7 matmuls
                can run before head 7's block s4 is normalized.
                """
                for hv in range(HPC):
                    rhs_t = ats[hv] if hv < HPC - 1 else (
                        at7 if at7 is not None else ats[HPC - 1])
                    nc.tensor.matmul(
                        psum, lhsT=wot[:, hv * 128 : (hv + 1) * 128],
                        rhs=rhs_t[:, s4 * 512 : (s4 + 1) * 512],
                        start=(hv == 0), stop=(hv == HPC - 1),
                    )
                ot = out_pool.tile([128, 512], B16, tag="ot",
                                   name=f"ot_{eo}_{s4}")
                nc.scalar.copy(out=ot, in_=psum)
                nc.sync.dma_start(
                    out=outt[eo * 128 : (eo + 1) * 128,
                             s4 * 512 : (s4 + 1) * 512],
                    in_=ot,
                )

            # Tail interleave: head 7's attention has no next-head Q-proj to
            # hide its exp latency behind, so feed the PE one Wo output-column
            # chain per block from the freed q-proj psum banks (head 7's
            # contribution joins as soon as each block of at_7 is normalized).
            # Only eo=0 is used so the second w_o pool slot stays free for
            # eo=1's weight DMA to proceed during the tail.
            wot0 = load_wo(0)
            at7 = make_at(HPC - 1)
            ats_t[HPC - 1] = at7
            qh7 = qhs.pop(HPC - 1)
            for b in (3, 2, 1, 0):
                attention(HPC - 1, qh7, at7, pt_pool, rc_pool, s4_range=(b,))
                wps = ps_pool.tile([128, 512], F32, tag="p1", bufs=2,
                                   name=f"pwot_0_{b}")
                wo_emit(wot0, wps, 0, b, at7=at7)
            ats.append(at7)

        # ---------------- Phase 3: Wo partial product ----------------
        with tc.tile_pool(name="wops", bufs=6, space="PSUM") as wops:
            for eo in range(1, NT_D):
                wot = load_wo(eo)
                psums = [
                    wops.tile([128, 512], F32, tag="pwo", name=f"pwo_{eo}_{s4}")
                    for s4 in range(NT_S4)
                ]
                last = eo == NT_D - 1
                if last:
                    # s4-outer: each psum completes 1/4 into this eo's work,
                    # so the final evacuations overlap the remaining matmuls
                    # instead of trailing the kernel.
                    for s4 in range(NT_S4):
                        for hv in range(HPC):
                            nc.tensor.matmul(
                                psums[s4],
                                lhsT=wot[:, hv * 128 : (hv + 1) * 128],
                                rhs=ats[hv][:, s4 * 512 : (s4 + 1) * 512],
                                start=(hv == 0), stop=(hv == HPC - 1),
                            )
                        ot = out_pool.tile([128, 512], B16, tag="ot",
                                           name=f"ot_{eo}_{s4}")
                        nc.scalar.copy(out=ot, in_=psums[s4])
                        nc.sync.dma_start(
                            out=outt[eo * 128 : (eo + 1) * 128,
                                     s4 * 512 : (s4 + 1) * 512],
                            in_=ot,
                        )
                    continue
                for hv in range(HPC):
                    lhsT = wot[:, hv * 128 : (hv + 1) * 128]
                    for s4 in range(NT_S4):
                        nc.tensor.matmul(
                            psums[s4], lhsT=lhsT,
                            rhs=ats[hv][:, s4 * 512 : (s4 + 1) * 512],
                            start=(hv == 0), stop=(hv == HPC - 1),
                        )
                for s4 in range(NT_S4):
                    ot = out_pool.tile([128, 512], B16, tag="ot",
                                       name=f"ot_{eo}_{s4}")
                    nc.scalar.copy(out=ot, in_=psums[s4])
                    nc.sync.dma_start(
                        out=outt[eo * 128 : (eo + 1) * 128,
                                 s4 * 512 : (s4 + 1) * 512],
                        in_=ot,
                    )

    nc.compile()
    _PROGRAM_CACHE["nc"] = nc
    return nc


def _host_prep(x, Wq, Wk, Wv, Wo):
    """Shard + lay out inputs for the 8 cores. Returns list of in_maps."""
    # Within-head permutation: [even dk indices, odd dk indices] so the RoPE
    # pair (2i, 2i+1) becomes (row i, row 64+i) of each head's 128-row block.
    perm1 = np.concatenate([np.arange(0, DK, 2), np.arange(1, DK, 2)])
    perm = np.concatenate([h * DK + perm1 for h in range(H)])

    scale = 1.0 / math.sqrt(DK)
    WqP = (Wq * scale)[perm]          # fold 1/sqrt(dk) into Q
    WkP = Wk[perm]

    # RoPE tables in the permuted feature-major layout [128, S].
    inv_freq = 1.0 / (ROPE_THETA ** (np.arange(0, DK, 2, dtype=np.float64) / DK))
    ang = inv_freq[:, None] * np.arange(S, dtype=np.float64)[None, :]  # [64, S]
    cosP = np.vstack([np.cos(ang), np.cos(ang)]).astype(BF16)
    sinP = np.vstack([-np.sin(ang), np.sin(ang)]).astype(BF16)

    # Causal 0/1 masks for the 4 diagonal-tile offsets: valid iff 128r+i <= j.
    i_idx = np.arange(128)[None, :, None]
    j_idx = np.arange(512)[None, None, :]
    r_idx = np.arange(4)[:, None, None]
    masks = np.ascontiguousarray(
        ((i_idx <= j_idx).astype(BF16))[0]
    )  # [128, 512] -- only the r=0 pattern is needed (diagonal narrowing)

    ones = np.ones((128, 128), dtype=BF16)

    def lhsT_blocks(Wt, n_out_tiles):
        # Wt: [contraction, width] (feature-major).
        # -> [n_out_tiles, 128, (contraction//128)*128]: per out-tile, a
        # partition-major contiguous block whose d-th 128-col slice is the
        # lhsT tile for contraction tile d (so each load is one linear DMA).
        kt = Wt.shape[0] // 128
        width = Wt.shape[1]
        blk = Wt.reshape(kt, 128, n_out_tiles, width // n_out_tiles)
        return np.ascontiguousarray(
            blk.transpose(2, 1, 0, 3).reshape(n_out_tiles, 128, kt * 128)
        ).astype(BF16)

    per_group = []
    for g in range(2):
        rows = slice(g * E, (g + 1) * E)
        wq_b = lhsT_blocks(WqP[rows].T, HPC)
        wk_b = lhsT_blocks(WkP[rows].T, HPC)
        wv_b = np.ascontiguousarray(
            Wv[rows].T.reshape(NT_D, 128, E)
        ).astype(BF16)
        # WoT [E, D]: lhsT blocks are [dv, e_out] tiles.
        wo_b = lhsT_blocks(np.ascontiguousarray(Wo[:, rows].T), NT_D)
        per_group.append((wq_b, wk_b, wv_b, wo_b))

    xts = []
    for b in range(B):
        xts.append(
            np.ascontiguousarray(x[b].T).astype(BF16).reshape(NT_D, 128, S)
        )

    in_maps = []
    for c in range(8):
        b, g = c // 2, c % 2
        wq_b, wk_b, wv_b, wo_b = per_group[g]
        in_maps.append(
            {
                "xt": xts[b],
                "wq": wq_b,
                "wk": wk_b,
                "wv": wv_b,
                "wo": wo_b,
                "cos": cosP,
                "sin": sinP,
                "msk": masks,
                "ones": ones,
            }
        )
    return in_maps


def kernel(x, Wq, Wk, Wv, Wo):
    global LAST_RESULT
    x = np.asarray(x, dtype=np.float32)
    Wq = np.asarray(Wq, dtype=np.float32)
    Wk = np.asarray(Wk, dtype=np.float32)
    Wv = np.asarray(Wv, dtype=np.float32)
    Wo = np.asarray(Wo, dtype=np.float32)

    if TRACE:
        _install_ntff_hook()

    from concourse.bass_utils import run_bass_kernel_spmd

    nc = _build_program()
    in_maps = _host_prep(x, Wq, Wk, Wv, Wo)
    res = run_bass_kernel_spmd(nc, in_maps, list(range(8)), trace=TRACE)
    LAST_RESULT = res

    out = np.empty((B, S, D), dtype=np.float32)
    for b in range(B):
        part = (
            res.results[2 * b]["outt"].astype(np.float32)
            + res.results[2 * b + 1]["outt"].astype(np.float32)
        )
        out[b] = part.T
    return out



# revision 28
# speedup vs baseline: 1.0167x; 1.0167x over previous
"""Causal multi-head attention (B=4, S=2048, D=2048, H=16, RoPE) on 8 TRN2 NeuronCores.

Sharding: core c handles (batch b = c//2, head-group g = c%2) -- 8 heads per core.
Each core computes its head-group's Q/K/V projections (column-sharded weights),
RoPE, causal softmax attention, and the row-sharded Wo partial product.
The host sums the two partial outputs per batch (the "all-reduce") and
transposes back.

Device-side layout is fully transposed ("feature-major"): activations are kept
as [feature, seq] so every GEMM contracts over the partition dimension without
any on-device transposes. All matmul inputs are bf16 (fp32 accumulate in PSUM);
softmax runs in fp32.

Schedule (single core): K-proj -> V-proj -> per-head software pipeline
[Q-proj(h) || attention(h-1)] -> Wo. The pipeline keeps the PE dense while
the RoPE/exp epilogues of the previous head run on Vector/Scalar.
"""

import math
import sys
import types

import numpy as np
import ml_dtypes

BF16 = ml_dtypes.bfloat16

S = 2048
D = 2048
H = 16
DK = 128
B = 4
E = 1024          # head-group width (8 heads x 128)
HPC = 8           # heads per core
NT_D = 16         # 128-wide tiles along the contraction (model) dim
NT_S4 = 4         # 512-wide tiles along seq
NT_S16 = 16       # 128-wide tiles along seq
ROPE_THETA = 10000.0

# Set by test harness to capture a profile; kernel() then stores results here.
TRACE = False
LAST_RESULT = None

_PROGRAM_CACHE = {}


def _install_ntff_hook():
    """Register the NTFF profile hook that this image's antenv lacks.

    Only needed when TRACE=True; degrades silently if the axon .so predates
    NRT profiling.
    """
    if "antenv.axon_hooks" in sys.modules:
        return
    holder = {"hook": None}
    mod = types.ModuleType("antenv.axon_hooks")
    mod.set_axon_ntff_profile_hook = lambda h: holder.__setitem__("hook", h)
    mod.get_axon_ntff_profile_hook = lambda: holder["hook"]
    sys.modules["antenv.axon_hooks"] = mod
    try:
        from trn_agent_boot.trn_boot import _ntff_profile_via_ctypes

        mod.set_axon_ntff_profile_hook(
            _ntff_profile_via_ctypes("/opt/axon/libaxon_pjrt.so")
        )
    except Exception:
        pass


def _build_program():
    """Build + compile the single-core Bass program (same program on all 8 cores)."""
    if "nc" in _PROGRAM_CACHE:
        return _PROGRAM_CACHE["nc"]

    from contextlib import ExitStack

    import concourse.mybir as mybir
    import concourse.tile as tile
    from concourse import bacc

    F32 = mybir.dt.float32
    B16 = mybir.dt.bfloat16

    nc = bacc.Bacc("TRN2", target_bir_lowering=False, debug=False, num_devices=8)

    F16 = mybir.dt.float16

    xt = nc.dram_tensor("xt", [NT_D, 128, S], B16, kind="ExternalInput").ap()
    # Weights are host-prepped into partition-major contiguous blocks so every
    # load is a single 4KB-per-partition DMA (256B-element strided loads were
    # rate-limiting the startup).
    wq = nc.dram_tensor("wq", [HPC, 128, NT_D * 128], B16, kind="ExternalInput").ap()
    wk = nc.dram_tensor("wk", [HPC, 128, NT_D * 128], B16, kind="ExternalInput").ap()
    wv = nc.dram_tensor("wv", [NT_D, 128, E], B16, kind="ExternalInput").ap()
    wo = nc.dram_tensor("wo", [NT_D, 128, HPC * 128], B16, kind="ExternalInput").ap()
    cos = nc.dram_tensor("cos", [128, S], B16, kind="ExternalInput").ap()
    sin = nc.dram_tensor("sin", [128, S], B16, kind="ExternalInput").ap()
    msk = nc.dram_tensor("msk", [128, 512], B16, kind="ExternalInput").ap()
    ones = nc.dram_tensor("ones", [128, 128], B16, kind="ExternalInput").ap()
    # bf16 partial-output: the host sums the two per-batch partials in fp32;
    # the bf16 rounding of the partials is well inside the error budget and
    # halves the output DMA traffic.
    outt = nc.dram_tensor("outt", [D, S], B16, kind="ExternalOutput").ap()

    Exp = mybir.ActivationFunctionType.Exp

    with tile.TileContext(nc, pool_alloc_mode="queue") as tc, ExitStack() as ctx:
        cpool = ctx.enter_context(tc.tile_pool(name="const", bufs=1))

        # Persistent activation stores (bf16).
        kt_pool = ctx.enter_context(tc.tile_pool(name="kt", bufs=HPC))
        v_pool = ctx.enter_context(tc.tile_pool(name="v", bufs=NT_S16))
        kts, vts, ats = [], [], []

        # PSUM: proj tag (2 bufs) coexists with attention tags (6 bufs) = 8.
        # Opened inside the phase-1/2 scope below (closed before the Wo phase).
        ps_pool = None

        def load_w(wdram, e, nm, w_pool):
            wt = w_pool.tile([128, NT_D * 128], B16, tag="w", name=f"w{nm}_{e}")
            nc.sync.dma_start(out=wt, in_=wdram[e])
            return wt

        def rope_epilogue(ps, qh, s4, nm, e, r_pool):
            """RoPE: qh[:, sl] = ps*cos + swap_halves(ps)*sin.

            ACT does only the partition-crossing half-swap (DVE is
            lane-locked); the cos-mul reads the PSUM directly on DVE.
            """
            sl = slice(s4 * 512, (s4 + 1) * 512)
            qs = r_pool.tile([128, 512], B16, tag="u", name=f"qs_{nm}_{e}_{s4}")
            nc.scalar.copy(out=qs[0:64, :], in_=ps[64:128, :])
            nc.scalar.copy(out=qs[64:128, :], in_=ps[0:64, :])
            nc.vector.tensor_mul(out=qh[:, sl], in0=ps, in1=cos_t[:, sl])
            nc.vector.tensor_mul(out=qs, in0=qs, in1=sin_t[:, sl])
            nc.vector.tensor_add(out=qh[:, sl], in0=qh[:, sl], in1=qs)

        def proj_pair(pool, pbufs, wt, qh, e, nm, sp, r_pool):
            """Emit one 1024-wide column group (2 psums) of a Q/K projection."""
            if True:
                psums = [
                    pool.tile([128, 512], F32, tag="p1", bufs=pbufs,
                              name=f"p{nm}_{e}_{sp}_{s2}")
                    for s2 in range(2)
                ]
                for d in range(NT_D):
                    lhsT = wt[:, d * 128 : (d + 1) * 128]
                    for s2 in range(2):
                        s4 = 2 * sp + s2
                        nc.tensor.matmul(
                            psums[s2], lhsT=lhsT,
                            rhs=xts[d][:, s4 * 512 : (s4 + 1) * 512],
                            start=(d == 0), stop=(d == NT_D - 1),
                        )
                for s2 in range(2):
                    s4 = 2 * sp + s2
                    rope_epilogue(psums[s2], qh, s4, nm, e, r_pool)

        def proj_qk(wdram, e, nm, out_pool, r_pool, w_pool, wt=None,
                    pool=None, pbufs=3):
            """One head's Q^T or K^T projection + RoPE; returns the bf16 tile."""
            if wt is None:
                wt = load_w(wdram, e, nm, w_pool)
            qh = out_pool.tile([128, S], B16, tag=nm + "t", name=f"{nm}h_{e}")
            for sp in range(2):
                proj_pair(pool, pbufs, wt, qh, e, nm, sp, r_pool)
            return qh

        # Deferred denominator finalizers. Each attention s4-block queues a
        # closure (den matmul + reciprocal + normalize); it is emitted a few
        # matmuls into the NEXT tensor-engine chain so the den matmul's wait
        # on the DVE fold chain is hidden behind already-runnable matmuls.
        deferred = []

        def emit_deferred():
            while deferred:
                deferred.pop(0)()

        def attention(h, qh, at, pt_pool, rc_pool, s4_range=None):
            """Causal attention for head h into at ([dv=128, S] bf16).

            The softmax denominator: full-width exp tiles are folded
            pairwise on DVE into fsum (bf16 adds), so the PE streams ONE
            ones-matmul per 512-q block plus the three narrow diagonal
            tiles, instead of one per k-tile.
            """
            for s4 in (range(NT_S4) if s4_range is None else s4_range):
                nsk = 4 * s4 + 4
                sl = slice(s4 * 512, (s4 + 1) * 512)
                pat = ps_pool.tile([128, 512], F32, tag="pat", bufs=2,
                                   name=f"pat_{h}_{s4}")
                fsum = None
                dpts = []  # narrow diagonal pt tiles (r>=1): own den matmuls
                for sk in range(nsk):
                    if sk == 2:
                        emit_deferred()
                    # Diagonal tiles: columns j < 128*r are fully masked; skip
                    # them in all three matmuls. The first tile (sk==0) is
                    # always full width, so its start=True write covers the
                    # whole accumulation bank.
                    r = sk - 4 * s4
                    off = 128 * r if r > 0 else 0
                    w = 512 - off
                    psc_t = ps_pool.tile([128, 512], F32, tag="psc", bufs=2,
                                         name=f"psc_{h}_{s4}_{sk}")
                    psc = psc_t[:, 0:w]
                    nc.tensor.matmul(
                        psc,
                        lhsT=kts[h][:, sk * 128 : (sk + 1) * 128],
                        rhs=qh[:, s4 * 512 + off : (s4 + 1) * 512],
                        start=True, stop=True,
                    )
                    pt_t = pt_pool.tile([128, 512], B16, tag="pt",
                                        name=f"pt_{h}_{s4}_{sk}")
                    pt = pt_t[:, 0:w]
                    # Promote the exp+mask chain past the current head's
                    # RoPE epilogue ops in the Scalar/Vector streams: the
                    # attnV matmul needs the masked tile now, while RoPE
                    # results aren't consumed until the next head.
                    with tc.high_priority(offset=400):
                        nc.scalar.activation(out=pt, in_=psc, func=Exp)
                        if r >= 0:
                            nc.vector.tensor_mul(
                                out=pt, in0=pt, in1=msk_t[:, 0:w],
                            )
                    nc.tensor.matmul(
                        pat[:, off:512],
                        lhsT=vts[sk][:, h * 128 : (h + 1) * 128],
                        rhs=pt, start=(sk == 0), stop=(sk == nsk - 1),
                    )
                    # Denominator: fold full-width tiles on DVE; keep the
                    # narrow diagonal tiles for direct (cheap) den matmuls.
                    if r >= 1:
                        dpts.append((pt, off))
                    elif fsum is None:
                        fsum = pt
                    elif fsum is not None and sk == 1:
                        ns = fs_pool.tile([128, 512], B16, tag="fs", bufs=2,
                                          name=f"fs_{h}_{s4}")
                        nc.vector.tensor_add(out=ns, in0=fsum, in1=pt)
                        fsum = ns
                    else:
                        nc.vector.tensor_add(out=fsum, in0=fsum, in1=pt)

                def finalize(h=h, s4=s4, sl=sl, pat=pat, fsum=fsum, dpts=dpts):
                    den = ps_pool.tile([128, 512], F32, tag="den", bufs=2,
                                       name=f"den_{h}_{s4}")
                    nc.tensor.matmul(den, lhsT=one_t, rhs=fsum,
                                     start=True, stop=(not dpts))
                    for i, (pt, off) in enumerate(dpts):
                        nc.tensor.matmul(
                            den[:, off:512], lhsT=one_t, rhs=pt,
                            start=False, stop=(i == len(dpts) - 1),
                        )
                    rcb = rc_pool.tile([128, 512], F32, tag="rcb", bufs=1,
                                       name=f"rcb_{h}_{s4}")
                    nc.vector.reciprocal_approx_fast(out=rcb, in_=den)
                    nc.vector.tensor_mul(out=at[:, sl], in0=pat, in1=rcb)

                deferred.append(finalize)

        # ---------------- Phase 1a: load X^T, K projections ----------------
        with ExitStack() as p1ctx:
            xt_pool = p1ctx.enter_context(tc.tile_pool(name="xt", bufs=NT_D))
            w_pool = p1ctx.enter_context(tc.tile_pool(name="wst", bufs=3))
            r_pool = p1ctx.enter_context(tc.tile_pool(name="rope", bufs=2))
            # DMA priming order: tiny constants (warm-up data) first, then
            # K-head-0/1 weights, then X^T tiles with cos/sin/msk mid-stream.
            one_t = cpool.tile([128, 128], B16, tag="one", name="one_t")
            nc.sync.dma_start(out=one_t, in_=ones)
            wt0 = w_pool.tile([128, NT_D * 128], B16, tag="w", name="wk_0")
            for c in range(4):
                nc.sync.dma_start(
                    out=wt0[:, c * 512 : (c + 1) * 512],
                    in_=wk[0][:, c * 512 : (c + 1) * 512],
                )
            wt1 = load_w(wk, 1, "k", w_pool)
            xts = []
            for d in range(NT_D):
                xtile = xt_pool.tile([128, S], B16, tag="xt", name=f"xt_{d}")
                if d == 0:
                    for c in range(4):
                        nc.sync.dma_start(
                            out=xtile[:, c * 512 : (c + 1) * 512],
                            in_=xt[0][:, c * 512 : (c + 1) * 512],
                        )
                else:
                    nc.sync.dma_start(out=xtile, in_=xt[d])
                xts.append(xtile)
                if d == 7:
                    cos_t = cpool.tile([128, S], B16, tag="cos", name="cos_t")
                    nc.sync.dma_start(out=cos_t, in_=cos)
                    sin_t = cpool.tile([128, S], B16, tag="sin", name="sin_t")
                    nc.sync.dma_start(out=sin_t, in_=sin)
                    msk_t = cpool.tile([128, 512], B16, tag="msk",
                                       name="msk_t")
                    nc.sync.dma_start(out=msk_t, in_=msk)

            # Prefetch K weights for heads 2/3 now: their DMAs queue behind
            # the x^T stream and arrive well before the head-2 projections
            # start (the bufs=2 schedule stalled ~3.6us at head 2 and
            # re-throttled the PE clock).
            wk_pre = {
                2: load_w(wk, 2, "k", w_pool),
                3: load_w(wk, 3, "k", w_pool),
            }

            kv_ps = tc.alloc_tile_pool(name="kvps", bufs=8, space="PSUM")
            # 7 interleaved psum groups (K head 0 fully + head 1 s4<3) so the
            # PE consumes each arriving x^T tile at ~1.5us/tile, matching the
            # DMA rate of the initial 8MB x^T load; head 1's s4=3 group runs
            # after the d-loop (8th bank) to bridge the epilogue drain.
            groups = [(0, 0), (0, 1), (0, 2), (0, 3), (1, 0), (1, 1), (1, 2)]
            ps01 = {
                g: kv_ps.tile([128, 512], F32, tag="p1", bufs=8,
                              name=f"pk_{g[0]}_{g[1]}")
                for g in groups
            }
            ps13 = kv_ps.tile([128, 512], F32, tag="p1", bufs=8, name="pk_1_3")
            # ~3.5us of junk matmuls on the ones tile: sustains PE activity
            # through the HAM SHORT window so the real projections run at
            # 2.4GHz instead of the cold 1.2GHz default.
            for i in range(72):
                nc.tensor.matmul(
                    ps01[(0, 0)][:, 0:128], lhsT=one_t, rhs=one_t,
                    start=(i == 0), stop=(i == 71), skip_group_check=True,
                )
            pre_w = {0: wt0, 1: wt1}
            for d in range(NT_D):
                for e, s4 in groups:
                    nc.tensor.matmul(
                        ps01[(e, s4)],
                        lhsT=pre_w[e][:, d * 128 : (d + 1) * 128],
                        rhs=xts[d][:, s4 * 512 : (s4 + 1) * 512],
                        start=(d == 0), stop=(d == NT_D - 1),
                    )
            for d in range(NT_D):
                nc.tensor.matmul(
                    ps13, lhsT=wt1[:, d * 128 : (d + 1) * 128],
                    rhs=xts[d][:, 3 * 512 : 4 * 512],
                    start=(d == 0), stop=(d == NT_D - 1),
                )
            kh0 = kt_pool.tile([128, S], B16, tag="kt", name="kh_0")
            kh1 = kt_pool.tile([128, S], B16, tag="kt", name="kh_1")
            for e, s4 in groups:
                rope_epilogue(ps01[(e, s4)], kh0 if e == 0 else kh1,
                              s4, "k", e, r_pool)
            rope_epilogue(ps13, kh1, 3, "k", 1, r_pool)
            kts.append(kh0)
            kts.append(kh1)
            for e in range(2, HPC):
                if e + 2 < HPC:
                    wk_pre[e + 2] = load_w(wk, e + 2, "k", w_pool)
                kts.append(proj_qk(wk, e, "k", kt_pool, r_pool, w_pool,
                                   wt=wk_pre.pop(e), pool=kv_ps, pbufs=8))

            # ------------- Phase 1b: V projection -------------
            with tc.tile_pool(name="wvp", bufs=NT_D) as wv_pool:
                wvts = []
                for d in range(NT_D):
                    wvt = wv_pool.tile([128, E], B16, tag="wv", name=f"wv_{d}")
                    nc.sync.dma_start(out=wvt, in_=wv[d])
                    wvts.append(wvt)
                # Prefetch the first two Q-projection weights during the V
                # phase so phase 2 starts without a weight-DMA stall.
                wq_pre = {
                    0: load_w(wq, 0, "q", w_pool),
                    1: load_w(wq, 1, "q", w_pool),
                }
                for s in range(NT_S16):
                    pv = [
                        kv_ps.tile([128, 512], F32, tag="p1", bufs=8,
                                   name=f"pv_{s}_{i}")
                        for i in range(2)
                    ]
                    for d in range(NT_D):
                        lhsT = xts[d][:, s * 128 : (s + 1) * 128]
                        for i in range(2):
                            nc.tensor.matmul(
                                pv[i], lhsT=lhsT,
                                rhs=wvts[d][:, i * 512 : (i + 1) * 512],
                                start=(d == 0), stop=(d == NT_D - 1),
                            )
                    vt = v_pool.tile([128, E], B16, tag="vt", name=f"vt_{s}")
                    nc.scalar.copy(out=vt[:, 0:512], in_=pv[0])
                    nc.scalar.copy(out=vt[:, 512:1024], in_=pv[1])
                    vts.append(vt)

            # ------- Phase 2: per-head pipeline: Q-proj(h) + attn(h-1) -------
            kv_ps.release()
            ps_pool = p1ctx.enter_context(
                tc.tile_pool(name="ps", bufs=2, space="PSUM")
            )
            attn_pool = ctx.enter_context(
                tc.tile_pool(name="attn", bufs=HPC, side="right")
            )
            qt_pool = p1ctx.enter_context(tc.tile_pool(name="qt", bufs=2))
            pt_pool = p1ctx.enter_context(tc.tile_pool(name="pt", bufs=6))
            rc_pool = p1ctx.enter_context(tc.tile_pool(name="rcp", bufs=2))
            fs_pool = rc_pool  # fold-sum tiles share the rcp ring slot

            def make_at(h):
                return attn_pool.tile([128, S], B16, tag="at", name=f"at_{h}")

            qhs = {}
            ats_t = {}
            for h in range(HPC):
                # Interleave the two q-proj column groups of head h with the
                # two attention halves of head h-1 so attention matmuls cover
                # the RoPE epilogue latency.
                if h + 2 < HPC:
                    wq_pre[h + 2] = load_w(wq, h + 2, "q", w_pool)
                wt = wq_pre.pop(h)
                qh = qt_pool.tile([128, S], B16, tag="qt", name=f"qh_{h}")
                qhs[h] = qh

                def q_group(s4):
                    ps = ps_pool.tile([128, 512], F32, tag="p1", bufs=2,
                                      name=f"pq_{h}_{s4}")
                    for d in range(NT_D):
                        nc.tensor.matmul(
                            ps, lhsT=wt[:, d * 128 : (d + 1) * 128],
                            rhs=xts[d][:, s4 * 512 : (s4 + 1) * 512],
                            start=(d == 0), stop=(d == NT_D - 1),
                        )
                        if d == 3:
                            emit_deferred()
                    rope_epilogue(ps, qh, s4, "q", h, r_pool)

                q_group(0)
                q_group(1)
                if h > 0:
                    # s4 order puts mask-free full tiles first so the DVE can
                    # drain the RoPE queue before mask-muls are needed.
                    ats_t[h - 1] = make_at(h - 1)
                    attention(h - 1, qhs[h - 1], ats_t[h - 1], pt_pool,
                              rc_pool, s4_range=(1, 0))
                q_group(2)
                q_group(3)
                if h > 0:
                    attention(h - 1, qhs.pop(h - 1), ats_t[h - 1], pt_pool,
                              rc_pool, s4_range=(3, 2))
                    ats.append(ats_t[h - 1])
            wo_pool = ctx.enter_context(
                tc.tile_pool(name="wop", bufs=2, side="right")
            )

            def load_wo(eo):
                wot = wo_pool.tile([128, HPC * 128], B16, tag="wo",
                                   name=f"wo_{eo}")
                nc.sync.dma_start(out=wot, in_=wo[eo])
                return wot

            out_pool = ctx.enter_context(
                tc.tile_pool(name="outp", bufs=2, side="right")
            )

            def wo_emit(wot, psum, eo, s4, at7=None):
                """One (eo, s4) Wo accumulation chain + evacuate + DMA out.

                at7: head-7's at tile while it is still being produced (tail
                interleave); head 7 is accumulated LAST so the first 7 matmuls
                can run before head 7's block s4 is normalized.
                """
                for hv in range(HPC):
                    rhs_t = ats[hv] if hv < HPC - 1 else (
                        at7 if at7 is not None else ats[HPC - 1])
                    nc.tensor.matmul(
                        psum, lhsT=wot[:, hv * 128 : (hv + 1) * 128],
                        rhs=rhs_t[:, s4 * 512 : (s4 + 1) * 512],
                        start=(hv == 0), stop=(hv == HPC - 1),
                    )
                    if hv == 3:
                        emit_deferred()
                ot = out_pool.tile([128, 512], B16, tag="ot",
                                   name=f"ot_{eo}_{s4}")
                nc.scalar.copy(out=ot, in_=psum)
                nc.sync.dma_start(
                    out=outt[eo * 128 : (eo + 1) * 128,
                             s4 * 512 : (s4 + 1) * 512],
                    in_=ot,
                )

            # Tail interleave: head 7's attention has no next-head Q-proj to
            # hide its exp latency behind, so feed the PE one Wo output-column
            # chain per block from the freed q-proj psum banks (head 7's
            # contribution joins as soon as each block of at_7 is normalized).
            # Only eo=0 is used so the second w_o pool slot stays free for
            # eo=1's weight DMA to proceed during the tail.
            wot0 = load_wo(0)
            at7 = make_at(HPC - 1)
            ats_t[HPC - 1] = at7
            qh7 = qhs.pop(HPC - 1)
            for b in (3, 2, 1, 0):
                attention(HPC - 1, qh7, at7, pt_pool, rc_pool, s4_range=(b,))
                wps = ps_pool.tile([128, 512], F32, tag="p1", bufs=2,
                                   name=f"pwot_0_{b}")
                wo_emit(wot0, wps, 0, b, at7=at7)
            ats.append(at7)
            emit_deferred()

        # ---------------- Phase 3: Wo partial product ----------------
        with tc.tile_pool(name="wops", bufs=6, space="PSUM") as wops:
            wot_next = load_wo(1)
            for eo in range(1, NT_D):
                wot = wot_next
                if eo + 1 < NT_D:
                    wot_next = load_wo(eo + 1)
                psums = [
                    wops.tile([128, 512], F32, tag="pwo", name=f"pwo_{eo}_{s4}")
                    for s4 in range(NT_S4)
                ]
                last = eo == NT_D - 1
                if last:
                    # s4-outer: each psum completes 1/4 into this eo's work,
                    # so the final evacuations overlap the remaining matmuls
                    # instead of trailing the kernel.
                    for s4 in range(NT_S4):
                        for hv in range(HPC):
                            nc.tensor.matmul(
                                psums[s4],
                                lhsT=wot[:, hv * 128 : (hv + 1) * 128],
                                rhs=ats[hv][:, s4 * 512 : (s4 + 1) * 512],
                                start=(hv == 0), stop=(hv == HPC - 1),
                            )
                        ot = out_pool.tile([128, 512], B16, tag="ot",
                                           name=f"ot_{eo}_{s4}")
                        nc.scalar.copy(out=ot, in_=psums[s4])
                        nc.sync.dma_start(
                            out=outt[eo * 128 : (eo + 1) * 128,
                                     s4 * 512 : (s4 + 1) * 512],
                            in_=ot,
                        )
                    continue
                for hv in range(HPC):
                    lhsT = wot[:, hv * 128 : (hv + 1) * 128]
                    for s4 in range(NT_S4):
                        nc.tensor.matmul(
                            psums[s4], lhsT=lhsT,
                            rhs=ats[hv][:, s4 * 512 : (s4 + 1) * 512],
                            start=(hv == 0), stop=(hv == HPC - 1),
                        )
                for s4 in range(NT_S4):
                    ot = out_pool.tile([128, 512], B16, tag="ot",
                                       name=f"ot_{eo}_{s4}")
                    nc.scalar.copy(out=ot, in_=psums[s4])
                    nc.sync.dma_start(
                        out=outt[eo * 128 : (eo + 1) * 128,
                                 s4 * 512 : (s4 + 1) * 512],
                        in_=ot,
                    )

    nc.compile()
    _PROGRAM_CACHE["nc"] = nc
    return nc


def _host_prep(x, Wq, Wk, Wv, Wo):
    """Shard + lay out inputs for the 8 cores. Returns list of in_maps."""
    # Within-head permutation: [even dk indices, odd dk indices] so the RoPE
    # pair (2i, 2i+1) becomes (row i, row 64+i) of each head's 128-row block.
    perm1 = np.concatenate([np.arange(0, DK, 2), np.arange(1, DK, 2)])
    perm = np.concatenate([h * DK + perm1 for h in range(H)])

    scale = 1.0 / math.sqrt(DK)
    WqP = (Wq * scale)[perm]          # fold 1/sqrt(dk) into Q
    WkP = Wk[perm]

    # RoPE tables in the permuted feature-major layout [128, S].
    inv_freq = 1.0 / (ROPE_THETA ** (np.arange(0, DK, 2, dtype=np.float64) / DK))
    ang = inv_freq[:, None] * np.arange(S, dtype=np.float64)[None, :]  # [64, S]
    cosP = np.vstack([np.cos(ang), np.cos(ang)]).astype(BF16)
    sinP = np.vstack([-np.sin(ang), np.sin(ang)]).astype(BF16)

    # Causal 0/1 masks for the 4 diagonal-tile offsets: valid iff 128r+i <= j.
    i_idx = np.arange(128)[None, :, None]
    j_idx = np.arange(512)[None, None, :]
    r_idx = np.arange(4)[:, None, None]
    masks = np.ascontiguousarray(
        ((i_idx <= j_idx).astype(BF16))[0]
    )  # [128, 512] -- only the r=0 pattern is needed (diagonal narrowing)

    ones = np.ones((128, 128), dtype=BF16)

    def lhsT_blocks(Wt, n_out_tiles):
        # Wt: [contraction, width] (feature-major).
        # -> [n_out_tiles, 128, (contraction//128)*128]: per out-tile, a
        # partition-major contiguous block whose d-th 128-col slice is the
        # lhsT tile for contraction tile d (so each load is one linear DMA).
        kt = Wt.shape[0] // 128
        width = Wt.shape[1]
        blk = Wt.reshape(kt, 128, n_out_tiles, width // n_out_tiles)
        return np.ascontiguousarray(
            blk.transpose(2, 1, 0, 3).reshape(n_out_tiles, 128, kt * 128)
        ).astype(BF16)

    per_group = []
    for g in range(2):
        rows = slice(g * E, (g + 1) * E)
        wq_b = lhsT_blocks(WqP[rows].T, HPC)
        wk_b = lhsT_blocks(WkP[rows].T, HPC)
        wv_b = np.ascontiguousarray(
            Wv[rows].T.reshape(NT_D, 128, E)
        ).astype(BF16)
        # WoT [E, D]: lhsT blocks are [dv, e_out] tiles.
        wo_b = lhsT_blocks(np.ascontiguousarray(Wo[:, rows].T), NT_D)
        per_group.append((wq_b, wk_b, wv_b, wo_b))

    xts = []
    for b in range(B):
        xts.append(
            np.ascontiguousarray(x[b].T).astype(BF16).reshape(NT_D, 128, S)
        )

    in_maps = []
    for c in range(8):
        b, g = c // 2, c % 2
        wq_b, wk_b, wv_b, wo_b = per_group[g]
        in_maps.append(
            {
                "xt": xts[b],
                "wq": wq_b,
                "wk": wk_b,
                "wv": wv_b,
                "wo": wo_b,
                "cos": cosP,
                "sin": sinP,
                "msk": masks,
                "ones": ones,
            }
        )
    return in_maps


def kernel(x, Wq, Wk, Wv, Wo):
    global LAST_RESULT
    x = np.asarray(x, dtype=np.float32)
    Wq = np.asarray(Wq, dtype=np.float32)
    Wk = np.asarray(Wk, dtype=np.float32)
    Wv = np.asarray(Wv, dtype=np.float32)
    Wo = np.asarray(Wo, dtype=np.float32)

    if TRACE:
        _install_ntff_hook()

    from concourse.bass_utils import run_bass_kernel_spmd

    nc = _build_program()
    in_maps = _host_prep(x, Wq, Wk, Wv, Wo)
    res = run_bass_kernel_spmd(nc, in_maps, list(range(8)), trace=TRACE)
    LAST_RESULT = res

    out = np.empty((B, S, D), dtype=np.float32)
    for b in range(B):
        part = (
            res.results[2 * b]["outt"].astype(np.float32)
            + res.results[2 * b + 1]["outt"].astype(np.float32)
        )
        out[b] = part.T
    return out

